# revision 1
# baseline (speedup 1.0000x reference)
"""Trainium2 Bass kernel for nn_Attention_3633542332637 (linear/cosine attention).

Math (per batch n):
  q = x @ Wq.T ; k = x @ Wk.T ; v = x @ Wv.T          (S=4096, D=1024, H=16, HD=64)
  q,k L2-normalized per head over HD; k,v masked; v /= mask.sum()**sigmoid(nc)
  kv_h = k_h^T @ v_h  (64x64) ; attn_h = q_h @ kv_h ; out = attn @ Wo.T

Sharding: core c = 2n + j handles batch n = c//2, sequence half j = c%2.
k/v are projected over the FULL sequence on both cores of a pair (the kv
reduction needs all positions; duplicating this beats the measured ~185us
latency of a pairwise AllReduce), q/attn/out over the local half only. The
host permutes each core's x.T to [partner half | own half] so a single SPMD
program can treat chunks 4..7 as "local".

Loop A fuses the k/v/q projections per 512-column chunk of x.T (k/v on all
chunks, q on local chunks), accumulates per-head kv outer products in PSUM,
and spills q-hat to DRAM (fp32r). Loop B re-reads q-hat, applies the tiny
per-head kv matmuls and the output projection.

All big matmuls run fp32r (full PE rate, ~8e-4 max rel err); kv runs fp32.
Host prep: x[n] and weights are pre-transposed (fp32 has no DMA transpose on
TRN2); mask / denominator / L2-norm factors fold into one [t, h] broadcast
multiply on k.
"""

import numpy as np

import concourse.bass as bass
import concourse.mybir as mybir
import concourse.tile as tile
from concourse import bacc
from concourse.bass_utils import run_bass_kernel_spmd

N, S, D = 4, 4096, 1024
H, HD = 16, 64
P = 128
DC = D // P            # 8 contraction chunks
SLOC = S // 2          # 2048 local positions
NCHF = S // 512        # 8 full-sequence chunks
NCH = SLOC // 512      # 4 local chunks
NCORES = 8

F32 = mybir.dt.float32
F32R = mybir.dt.float32r
SQUARE = mybir.ActivationFunctionType.Square

_BUILD_CACHE = {}


def build(reps=1, phases="both"):
    key = ("nc", reps, phases)
    if key in _BUILD_CACHE:
        return _BUILD_CACHE[key]
    nc = bacc.Bacc("TRN2", target_bir_lowering=False, debug=False)

    # ---- I/O ----
    # x[n].T with columns permuted to [partner half | own half]
    xt = nc.declare_dram_parameter("xt", [D, S], F32, isOutput=False)
    wk = nc.declare_dram_parameter("wk", [D, D], F32, isOutput=False)   # Wk.T [d, e]
    wv = nc.declare_dram_parameter("wv", [D, D], F32, isOutput=False)
    wq = nc.declare_dram_parameter("wq", [D, D], F32, isOutput=False)
    wo = nc.declare_dram_parameter("wo", [D, D], F32, isOutput=False)
    # mvs[p, tt, h] = mask_perm[128*tt + p] * vscale[h]  (full permuted sequence)
    mvs = nc.declare_dram_parameter("mvs", [P, S // P, H], F32, isOutput=False)
    sel = nc.declare_dram_parameter("sel", [P, P], F32, isOutput=False)  # block-diag ones
    out = nc.declare_dram_parameter("out", [SLOC, D], F32, isOutput=True)

    def dram3(t, dt=F32R):
        ap = t.ap().rearrange("(dc p) x -> p dc x", p=P)
        if dt is not None and dt != F32:
            ap = ap.bitcast(dt)
        return ap

    with tile.TileContext(nc) as tc:
        with (
            tc.tile_pool(name="consts", bufs=1) as consts,
            tc.tile_pool(name="cdram", bufs=1, space="DRAM") as cdram,
        ):
            wq_sb = consts.tile([P, DC, D], F32R)
            wo_sb = consts.tile([P, DC, D], F32R)
            nc.sync.dma_start(out=wq_sb[:], in_=dram3(wq))
            nc.sync.dma_start(out=wo_sb[:], in_=dram3(wo))
            sel_sb = consts.tile([P, P], F32R)
            nc.sync.dma_start(out=sel_sb[:], in_=sel.ap().bitcast(F32R))
            mvs_sb = consts.tile([P, S // P, H], F32)
            nc.sync.dma_start(out=mvs_sb[:], in_=mvs.ap())
            kv_r = consts.tile([64, H, HD], F32R)
            xt3 = dram3(xt)
            # DRAM spill for q-hat: even-head rows and odd-head rows separately
            qh_dram_e = cdram.tile([NCH, DC, 64, 512], F32R)
            qh_dram_o = cdram.tile([NCH, DC, 64, 512], F32R)

            def emit_loop_a():
              with (
                tc.tile_pool(name="pAw", bufs=1) as pAw,
                tc.tile_pool(name="pAx", bufs=2) as pAx,
                tc.tile_pool(name="pAwork", bufs=2) as pAwork,
                tc.tile_pool(name="pAstats", bufs=4) as pAstats,
                tc.tile_pool(name="pApsum", bufs=4, space="PSUM") as pApsum,
                tc.tile_pool(name="pAqpsum", bufs=2, space="PSUM") as pAqpsum,
                tc.tile_pool(name="kvpool", bufs=1, space="PSUM") as kvpool,
              ):
                wk_sb = pAw.tile([P, DC, D], F32R)
                wv_sb = pAw.tile([P, DC, D], F32R)
                nc.sync.dma_start(out=wk_sb[:], in_=dram3(wk))
                nc.sync.dma_start(out=wv_sb[:], in_=dram3(wv))

                kv_ps = kvpool.tile([64, H * HD], F32)  # 2 banks, accumulated all loop

                for c in range(NCHF):
                    if phases == "Aq" and c < NCHF - NCH:
                        continue
                    x_sb = pAx.tile([P, DC, 512], F32R)
                    nc.sync.dma_start(out=x_sb[:], in_=xt3[:, :, 512 * c : 512 * (c + 1)])
                    # ---- k/v projection + kv accumulation, 4 t-tiles ----
                    for tt4 in range(4 if phases != "Aq" else 0):
                        tt = 4 * c + tt4
                        for half in range(2):   # e-halves == head blocks 0-7 / 8-15
                            h0 = 8 * half
                            kps = pApsum.tile([P, 512], F32, tag="proj")
                            for dc in range(DC):
                                nc.tensor.matmul(
                                    kps[:],
                                    lhsT=x_sb[:, dc, 128 * tt4 : 128 * (tt4 + 1)],
                                    rhs=wk_sb[:, dc, 512 * half : 512 * (half + 1)],
                                    start=(dc == 0),
                                    stop=(dc == DC - 1),
                                )
                            vps = pApsum.tile([P, 512], F32, tag="proj")
                            for dc in range(DC):
                                nc.tensor.matmul(
                                    vps[:],
                                    lhsT=x_sb[:, dc, 128 * tt4 : 128 * (tt4 + 1)],
                                    rhs=wv_sb[:, dc, 512 * half : 512 * (half + 1)],
                                    start=(dc == 0),
                                    stop=(dc == DC - 1),
                                )
                            # k normalization factors for these 8 heads
                            ksqf = pAstats.tile([P, 512], F32, tag="ksqf")
                            nc.scalar.activation(out=ksqf[:], in_=kps[:], func=SQUARE)
                            ksq = pAstats.tile([P, 8], F32, tag="ksq")
                            nc.vector.reduce_sum(
                                out=ksq[:],
                                in_=ksqf[:].rearrange("p (h a) -> p h a", h=8),
                                axis=mybir.AxisListType.X,
                            )
                            r = pAstats.tile([P, 8], F32, tag="r")
                            nc.scalar.sqrt(out=r[:], in_=ksq[:])
                            nc.vector.tensor_scalar_max(out=r[:], in0=r[:], scalar1=1e-12)
                            nc.vector.reciprocal(out=r[:], in_=r[:])
                            nc.vector.tensor_mul(
                                out=r[:], in0=r[:], in1=mvs_sb[:, tt, h0 : h0 + 8]
                            )
                            # khat = kps * r (psum in0, sbuf out), v copy to sbuf
                            khat = pAwork.tile([P, 8, HD], F32, tag="khat")
                            nc.vector.tensor_tensor(
                                khat[:],
                                kps[:].rearrange("p (h a) -> p h a", h=8),
                                r[:, :, None].to_broadcast((P, 8, HD)),
                                mybir.AluOpType.mult,
                            )
                            v_sb = pAwork.tile([P, 512], F32, tag="v_sb")
                            nc.scalar.copy(out=v_sb[:], in_=vps[:])
                            for hh in range(8):
                                h = h0 + hh
                                nc.tensor.matmul(
                                    kv_ps[:, HD * h : HD * (h + 1)],
                                    lhsT=khat[:, hh, :],
                                    rhs=v_sb[:, HD * hh : HD * (hh + 1)],
                                    start=(tt == 0 and hh == 0),
                                    stop=(tt == S // P - 1 and hh == 7),
                                    skip_group_check=True,
                                )
                    # ---- q projection + normalization + spill (local chunks) ----
                    if c >= NCHF - NCH and phases != "Akv":
                        cl = c - (NCHF - NCH)
                        for et in range(DC):
                            qps = pAqpsum.tile([P, 512], F32, tag="qn")
                            for dc in range(DC):
                                nc.tensor.matmul(
                                    qps[:],
                                    lhsT=wq_sb[:, dc, 128 * et : 128 * (et + 1)],
                                    rhs=x_sb[:, dc, :],
                                    start=(dc == 0),
                                    stop=(dc == DC - 1),
                                )
                            qsb = pAwork.tile([P, 512], F32, tag="qsb")
                            nc.scalar.copy(out=qsb[:], in_=qps[:])
                            q2 = pAwork.tile([P, 512], F32R, tag="q2")
                            nc.scalar.activation(out=q2[:], in_=qsb[:], func=SQUARE)
                            nps = pAqpsum.tile([P, 512], F32, tag="qn")
                            nc.tensor.matmul(nps[:], lhsT=sel_sb[:], rhs=q2[:], start=True, stop=True)
                            rn = pAwork.tile([P, 512], F32, tag="rn")
                            nc.scalar.sqrt(out=rn[:], in_=nps[:])
                            nc.vector.tensor_scalar_max(out=rn[:], in0=rn[:], scalar1=1e-12)
                            nc.vector.reciprocal(out=rn[:], in_=rn[:])
                            qh = pAwork.tile([P, 512], F32R, tag="qh")
                            nc.vector.tensor_mul(out=qh[:], in0=qsb[:], in1=rn[:])
                            nc.sync.dma_start(out=qh_dram_e[cl, et], in_=qh[0:64, :])
                            nc.sync.dma_start(out=qh_dram_o[cl, et], in_=qh[64:128, :])
                # kv (fp32 psum) -> sbuf as fp32r for the attn matmuls
                nc.vector.tensor_copy(
                    out=kv_r[:], in_=kv_ps[:].rearrange("p (h a) -> p h a", h=H)
                )

            def emit_loop_b():
              with (
                tc.tile_pool(name="pBq", bufs=2) as pBq,
                tc.tile_pool(name="pBat", bufs=2) as pBat,
                tc.tile_pool(name="pBout", bufs=3) as pBout,
                tc.tile_pool(name="pBpsum", bufs=2, space="PSUM") as pBpsum,
              ):
                for c in range(NCH):
                    qh_e = pBq.tile([64, DC, 512], F32R, tag="qh_e")
                    qh_o = pBq.tile([64, DC, 512], F32R, tag="qh_o")
                    nc.sync.dma_start(
                        out=qh_e[:], in_=qh_dram_e[c].rearrange("et p t -> p et t")
                    )
                    nc.sync.dma_start(
                        out=qh_o[:], in_=qh_dram_o[c].rearrange("et p t -> p et t")
                    )
                    at_sb = pBat.tile([P, DC, 512], F32R, tag="at_sb")
                    for h in range(H):
                        aps = pBpsum.tile([64, 512], F32, tag="aps")
                        qsrc = qh_e if h % 2 == 0 else qh_o
                        nc.tensor.matmul(
                            aps[:], lhsT=kv_r[:, h, :], rhs=qsrc[:, h // 2, :],
                            start=True, stop=True,
                        )
                        nc.vector.tensor_copy(
                            out=at_sb[64 * (h % 2) : 64 * (h % 2) + 64, h // 2, :],
                            in_=aps[:],
                        )
                    for tt4 in range(4):
                        o_sb = pBout.tile([P, D], F32, tag="o_sb")
                        for half in range(2):
                            ops = pBpsum.tile([P, 512], F32, tag="ops")
                            for ec in range(DC):
                                nc.tensor.matmul(
                                    ops[:],
                                    lhsT=at_sb[:, ec, 128 * tt4 : 128 * (tt4 + 1)],
                                    rhs=wo_sb[:, ec, 512 * half : 512 * (half + 1)],
                                    start=(ec == 0),
                                    stop=(ec == DC - 1),
                                )
                            nc.scalar.copy(out=o_sb[:, 512 * half : 512 * (half + 1)], in_=ops[:])
                        t0 = 512 * c + 128 * tt4
                        nc.sync.dma_start(out=out.ap()[t0 : t0 + P, :], in_=o_sb[:])

            if phases in ("B", "none", "Aq") or reps == 0:
                nc.vector.memset(kv_r[:].bitcast(F32), 0.0)
            for _rep in range(reps):
                if phases in ("both", "A", "Akv", "Aq"):
                    emit_loop_a()
                if phases in ("both", "B"):
                    emit_loop_b()

    nc.finalize()
    _BUILD_CACHE[key] = nc
    return nc


def _sel_np():
    e = np.arange(P)
    return (e[:, None] // HD == e[None, :] // HD).astype(np.float32)


def make_in_maps(x, mask, Wq, Wk, Wv, Wo, norm_const):
    x = np.asarray(x)
    mask = np.asarray(mask)
    Wq = np.asarray(Wq); Wk = np.asarray(Wk); Wv = np.asarray(Wv); Wo = np.asarray(Wo)
    norm_const = np.asarray(norm_const)

    wkT = np.ascontiguousarray(Wk.T)
    wvT = np.ascontiguousarray(Wv.T)
    wqT = np.ascontiguousarray(Wq.T)
    woT = np.ascontiguousarray(Wo.T)
    sel = _sel_np()

    m32 = mask.astype(np.float32)
    # denom[n, h] = mask[n].sum() ** sigmoid(norm_const[h]); vscale = 1/denom
    sig = 1.0 / (1.0 + np.exp(-norm_const.astype(np.float32).reshape(H)))
    msum = m32.sum(axis=1)  # [N]
    denom = msum[:, None] ** sig[None, :]  # [N, H]
    vscale = (1.0 / denom).astype(np.float32)

    in_maps = []
    xts = {n: np.ascontiguousarray(x[n].T) for n in range(N)}
    for c in range(NCORES):
        n, j = c // 2, c % 2
        other = 1 - j
        # permuted x.T: [partner half | own half]
        xp = np.concatenate(
            [xts[n][:, other * SLOC : (other + 1) * SLOC],
             xts[n][:, j * SLOC : (j + 1) * SLOC]], axis=1
        )
        mp = np.concatenate(
            [m32[n, other * SLOC : (other + 1) * SLOC],
             m32[n, j * SLOC : (j + 1) * SLOC]]
        )
        mcol = mp.reshape(S // P, P).T  # [p, tt]
        mvs = np.ascontiguousarray(
            mcol[:, :, None] * vscale[n][None, None, :]
        ).astype(np.float32)
        in_maps.append({
            "xt": np.ascontiguousarray(xp),
            "wk": wkT, "wv": wvT, "wq": wqT, "wo": woT,
            "mvs": mvs, "sel": sel,
        })
    return in_maps


def kernel(x, mask, Wq, Wk, Wv, Wo, norm_const):
    in_maps = make_in_maps(x, mask, Wq, Wk, Wv, Wo, norm_const)
    nc = build()
    res = run_bass_kernel_spmd(nc, in_maps, core_ids=list(range(NCORES)))
    out = np.empty((N, S, D), dtype=np.float32)
    for c in range(NCORES):
        n, j = c // 2, c % 2
        out[n, j * SLOC : (j + 1) * SLOC, :] = res.results[c]["out"]
    return out



# revision 4
# speedup vs baseline: 1.1254x; 1.1254x over previous
"""Trainium2 Bass kernel for nn_Attention_3633542332637 (linear/cosine attention).

Math (per batch n):
  q = x @ Wq.T ; k = x @ Wk.T ; v = x @ Wv.T          (S=4096, D=1024, H=16, HD=64)
  q,k L2-normalized per head over HD; k,v masked; v /= mask.sum()**sigmoid(nc)
  kv_h = k_h^T @ v_h  (64x64) ; attn_h = q_h @ kv_h ; out = attn @ Wo.T

Sharding: core c = 2n + j handles batch n = c//2, sequence half j = c%2.
k/v are projected over the FULL sequence on both cores of a pair (the kv
reduction needs all positions; duplicating this beats the measured latency
of a pairwise AllReduce), q/attn/out over the local half only. The host
permutes each core's x.T to [partner half | own half] so a single SPMD
program can treat chunks 4..7 as "local".

All matmuls run bf16 (the bf16 weight-load path is ~2-7x faster per matmul
than fp32r on TRN2; PSUM still accumulates fp32, rel err ~1e-3 vs the fp32
reference). Host prep: x[n] and weights pre-transposed and converted to
bf16; mask / denominator / L2-norm factors fold into one [t, h] broadcast
multiply on k.

Loop A fuses the k/v/q projections per 512-column chunk of x.T (k/v on all
chunks, q on local chunks), accumulates per-head kv outer products in PSUM,
and spills q-hat to DRAM (bf16). Loop B re-reads q-hat, applies the tiny
per-head kv matmuls and the output projection.
"""

import numpy as np

import concourse.bass as bass
import concourse.mybir as mybir
import concourse.tile as tile
from concourse import bacc
from concourse.bass_utils import run_bass_kernel_spmd

N, S, D = 4, 4096, 1024
H, HD = 16, 64
P = 128
DC = D // P            # 8 contraction chunks
SLOC = S // 2          # 2048 local positions
NCHF = S // 512        # 8 full-sequence chunks
NCH = SLOC // 512      # 4 local chunks
NCORES = 8

F32 = mybir.dt.float32
BF16 = mybir.dt.bfloat16
SQUARE = mybir.ActivationFunctionType.Square

_BUILD_CACHE = {}


def build(reps=1, phases="both"):
    key = ("nc", reps, phases)
    if key in _BUILD_CACHE:
        return _BUILD_CACHE[key]
    nc = bacc.Bacc("TRN2", target_bir_lowering=False, debug=False)

    # ---- I/O ----
    # x[n].T with columns permuted to [partner half | own half], bf16
    xt = nc.declare_dram_parameter("xt", [D, S], BF16, isOutput=False)
    wk = nc.declare_dram_parameter("wk", [D, D], BF16, isOutput=False)   # Wk.T [d, e]
    wv = nc.declare_dram_parameter("wv", [D, D], BF16, isOutput=False)
    wq = nc.declare_dram_parameter("wq", [D, D], BF16, isOutput=False)
    wo = nc.declare_dram_parameter("wo", [D, D], BF16, isOutput=False)
    # mvs[p, tt, h] = mask_perm[128*tt + p] * vscale[h]  (full permuted sequence)
    mvs = nc.declare_dram_parameter("mvs", [P, S // P, H], F32, isOutput=False)
    sel = nc.declare_dram_parameter("sel", [P, P], BF16, isOutput=False)  # block-diag ones
    out = nc.declare_dram_parameter("out", [SLOC, D], F32, isOutput=True)

    def dram3(t):
        return t.ap().rearrange("(dc p) x -> p dc x", p=P)

    with tile.TileContext(nc) as tc:
        with (
            tc.tile_pool(name="consts", bufs=1) as consts,
            tc.tile_pool(name="cdram", bufs=1, space="DRAM") as cdram,
        ):
            wq_sb = consts.tile([P, DC, D], BF16)
            wo_sb = consts.tile([P, DC, D], BF16)
            nc.sync.dma_start(out=wq_sb[:], in_=dram3(wq))
            nc.sync.dma_start(out=wo_sb[:], in_=dram3(wo))
            sel_sb = consts.tile([P, P], BF16)
            nc.sync.dma_start(out=sel_sb[:], in_=sel.ap())
            mvs_sb = consts.tile([P, S // P, H], F32)
            nc.sync.dma_start(out=mvs_sb[:], in_=mvs.ap())
            kv_r = consts.tile([64, H, HD], BF16)
            xt3 = dram3(xt)
            # DRAM spill for q-hat: even-head rows and odd-head rows separately
            qh_dram_e = cdram.tile([NCH, DC, 64, 512], BF16)
            qh_dram_o = cdram.tile([NCH, DC, 64, 512], BF16)

            def emit_loop_a():
              with (
                tc.tile_pool(name="pAw", bufs=1) as pAw,
                tc.tile_pool(name="pAx", bufs=2) as pAx,
                tc.tile_pool(name="pAwork", bufs=2) as pAwork,
                tc.tile_pool(name="pAstats", bufs=4) as pAstats,
                tc.tile_pool(name="pApsum", bufs=4, space="PSUM") as pApsum,
                tc.tile_pool(name="pAqpsum", bufs=2, space="PSUM") as pAqpsum,
                tc.tile_pool(name="kvpool", bufs=1, space="PSUM") as kvpool,
              ):
                wk_sb = pAw.tile([P, DC, D], BF16)
                wv_sb = pAw.tile([P, DC, D], BF16)
                nc.sync.dma_start(out=wk_sb[:], in_=dram3(wk))
                nc.sync.dma_start(out=wv_sb[:], in_=dram3(wv))

                kv_ps = kvpool.tile([64, H * HD], F32)  # 2 banks, accumulated all loop

                for c in range(NCHF):
                    if phases == "Aq" and c < NCHF - NCH:
                        continue
                    x_sb = pAx.tile([P, DC, 512], BF16)
                    nc.sync.dma_start(out=x_sb[:], in_=xt3[:, :, 512 * c : 512 * (c + 1)])
                    # ---- k/v projection + kv accumulation, 4 t-tiles ----
                    for tt4 in range(4 if phases != "Aq" else 0):
                        tt = 4 * c + tt4
                        for half in range(2):   # e-halves == head blocks 0-7 / 8-15
                            h0 = 8 * half
                            kps = pApsum.tile([P, 512], F32, tag="proj")
                            for dc in range(DC):
                                nc.tensor.matmul(
                                    kps[:],
                                    lhsT=x_sb[:, dc, 128 * tt4 : 128 * (tt4 + 1)],
                                    rhs=wk_sb[:, dc, 512 * half : 512 * (half + 1)],
                                    start=(dc == 0),
                                    stop=(dc == DC - 1),
                                )
                            vps = pApsum.tile([P, 512], F32, tag="proj")
                            for dc in range(DC):
                                nc.tensor.matmul(
                                    vps[:],
                                    lhsT=x_sb[:, dc, 128 * tt4 : 128 * (tt4 + 1)],
                                    rhs=wv_sb[:, dc, 512 * half : 512 * (half + 1)],
                                    start=(dc == 0),
                                    stop=(dc == DC - 1),
                                )
                            # k normalization factors for these 8 heads
                            ksqf = pAstats.tile([P, 512], BF16, tag="ksqf")
                            nc.scalar.activation(out=ksqf[:], in_=kps[:], func=SQUARE)
                            ksq = pAstats.tile([P, 8], F32, tag="ksq")
                            nc.vector.reduce_sum(
                                out=ksq[:],
                                in_=ksqf[:].rearrange("p (h a) -> p h a", h=8),
                                axis=mybir.AxisListType.X,
                            )
                            r = pAstats.tile([P, 8], F32, tag="r")
                            nc.scalar.sqrt(out=r[:], in_=ksq[:])
                            nc.vector.tensor_scalar_max(out=r[:], in0=r[:], scalar1=1e-12)
                            nc.vector.reciprocal(out=r[:], in_=r[:])
                            nc.vector.tensor_mul(
                                out=r[:], in0=r[:], in1=mvs_sb[:, tt, h0 : h0 + 8]
                            )
                            # khat = kps * r (psum in0, sbuf out), v copy to sbuf
                            khat = pAwork.tile([P, 8, HD], BF16, tag="khat")
                            nc.vector.tensor_tensor(
                                khat[:],
                                kps[:].rearrange("p (h a) -> p h a", h=8),
                                r[:, :, None].to_broadcast((P, 8, HD)),
                                mybir.AluOpType.mult,
                            )
                            v_sb = pAwork.tile([P, 512], BF16, tag="v_sb")
                            nc.scalar.copy(out=v_sb[:], in_=vps[:])
                            for hh in range(8):
                                h = h0 + hh
                                nc.tensor.matmul(
                                    kv_ps[:, HD * h : HD * (h + 1)],
                                    lhsT=khat[:, hh, :],
                                    rhs=v_sb[:, HD * hh : HD * (hh + 1)],
                                    start=(tt == 0 and hh == 0),
                                    stop=(tt == S // P - 1 and hh == 7),
                                    skip_group_check=True,
                                )
                    # ---- q projection + normalization + spill (local chunks) ----
                    if c >= NCHF - NCH and phases != "Akv":
                        cl = c - (NCHF - NCH)
                        for et in range(DC):
                            qps = pAqpsum.tile([P, 512], F32, tag="qn")
                            for dc in range(DC):
                                nc.tensor.matmul(
                                    qps[:],
                                    lhsT=wq_sb[:, dc, 128 * et : 128 * (et + 1)],
                                    rhs=x_sb[:, dc, :],
                                    start=(dc == 0),
                                    stop=(dc == DC - 1),
                                )
                            qsb = pAwork.tile([P, 512], BF16, tag="qsb")
                            nc.scalar.copy(out=qsb[:], in_=qps[:])
                            q2 = pAwork.tile([P, 512], BF16, tag="q2")
                            nc.scalar.activation(out=q2[:], in_=qsb[:], func=SQUARE)
                            nps = pAqpsum.tile([P, 512], F32, tag="qn")
                            nc.tensor.matmul(nps[:], lhsT=sel_sb[:], rhs=q2[:], start=True, stop=True)
                            rn = pAwork.tile([P, 512], F32, tag="rn")
                            nc.scalar.sqrt(out=rn[:], in_=nps[:])
                            nc.vector.tensor_scalar_max(out=rn[:], in0=rn[:], scalar1=1e-12)
                            nc.vector.reciprocal(out=rn[:], in_=rn[:])
                            qh = pAwork.tile([P, 512], BF16, tag="qh")
                            nc.vector.tensor_mul(out=qh[:], in0=qsb[:], in1=rn[:])
                            nc.sync.dma_start(out=qh_dram_e[cl, et], in_=qh[0:64, :])
                            nc.sync.dma_start(out=qh_dram_o[cl, et], in_=qh[64:128, :])
                # kv (fp32 psum) -> sbuf as bf16 for the attn matmuls
                nc.vector.tensor_copy(
                    out=kv_r[:], in_=kv_ps[:].rearrange("p (h a) -> p h a", h=H)
                )

            def emit_loop_b():
              with (
                tc.tile_pool(name="pBq", bufs=2) as pBq,
                tc.tile_pool(name="pBat", bufs=2) as pBat,
                tc.tile_pool(name="pBout", bufs=3) as pBout,
                tc.tile_pool(name="pBpsum", bufs=2, space="PSUM") as pBpsum,
              ):
                for c in range(NCH):
                    qh_e = pBq.tile([64, DC, 512], BF16, tag="qh_e")
                    qh_o = pBq.tile([64, DC, 512], BF16, tag="qh_o")
                    nc.sync.dma_start(
                        out=qh_e[:], in_=qh_dram_e[c].rearrange("et p t -> p et t")
                    )
                    nc.sync.dma_start(
                        out=qh_o[:], in_=qh_dram_o[c].rearrange("et p t -> p et t")
                    )
                    at_sb = pBat.tile([P, DC, 512], BF16, tag="at_sb")
                    for h in range(H):
                        aps = pBpsum.tile([64, 512], F32, tag="aps")
                        qsrc = qh_e if h % 2 == 0 else qh_o
                        nc.tensor.matmul(
                            aps[:], lhsT=kv_r[:, h, :], rhs=qsrc[:, h // 2, :],
                            start=True, stop=True,
                        )
                        nc.vector.tensor_copy(
                            out=at_sb[64 * (h % 2) : 64 * (h % 2) + 64, h // 2, :],
                            in_=aps[:],
                        )
                    for tt4 in range(4):
                        o_sb = pBout.tile([P, D], F32, tag="o_sb")
                        for half in range(2):
                            ops = pBpsum.tile([P, 512], F32, tag="ops")
                            for ec in range(DC):
                                nc.tensor.matmul(
                                    ops[:],
                                    lhsT=at_sb[:, ec, 128 * tt4 : 128 * (tt4 + 1)],
                                    rhs=wo_sb[:, ec, 512 * half : 512 * (half + 1)],
                                    start=(ec == 0),
                                    stop=(ec == DC - 1),
                                )
                            nc.scalar.copy(out=o_sb[:, 512 * half : 512 * (half + 1)], in_=ops[:])
                        t0 = 512 * c + 128 * tt4
                        nc.sync.dma_start(out=out.ap()[t0 : t0 + P, :], in_=o_sb[:])

            if phases in ("B", "none", "Aq") or reps == 0:
                nc.vector.memset(kv_r[:], 0.0)
            for _rep in range(reps):
                if phases in ("both", "A", "Akv", "Aq"):
                    emit_loop_a()
                if phases in ("both", "B"):
                    emit_loop_b()

    nc.finalize()
    _BUILD_CACHE[key] = nc
    return nc


def _sel_np():
    e = np.arange(P)
    return (e[:, None] // HD == e[None, :] // HD).astype(np.float32)


def make_in_maps(x, mask, Wq, Wk, Wv, Wo, norm_const):
    bf16 = mybir.dt.np(BF16)
    x = np.asarray(x)
    mask = np.asarray(mask)
    Wq = np.asarray(Wq); Wk = np.asarray(Wk); Wv = np.asarray(Wv); Wo = np.asarray(Wo)
    norm_const = np.asarray(norm_const)

    wkT = np.ascontiguousarray(Wk.T).astype(bf16)
    wvT = np.ascontiguousarray(Wv.T).astype(bf16)
    wqT = np.ascontiguousarray(Wq.T).astype(bf16)
    woT = np.ascontiguousarray(Wo.T).astype(bf16)
    sel = _sel_np().astype(bf16)

    m32 = mask.astype(np.float32)
    # denom[n, h] = mask[n].sum() ** sigmoid(norm_const[h]); vscale = 1/denom
    sig = 1.0 / (1.0 + np.exp(-norm_const.astype(np.float32).reshape(H)))
    msum = m32.sum(axis=1)  # [N]
    denom = msum[:, None] ** sig[None, :]  # [N, H]
    vscale = (1.0 / denom).astype(np.float32)

    in_maps = []
    xts = {n: np.ascontiguousarray(x[n].T).astype(bf16) for n in range(N)}
    for c in range(NCORES):
        n, j = c // 2, c % 2
        other = 1 - j
        # permuted x.T: [partner half | own half]
        xp = np.concatenate(
            [xts[n][:, other * SLOC : (other + 1) * SLOC],
             xts[n][:, j * SLOC : (j + 1) * SLOC]], axis=1
        )
        mp = np.concatenate(
            [m32[n, other * SLOC : (other + 1) * SLOC],
             m32[n, j * SLOC : (j + 1) * SLOC]]
        )
        mcol = mp.reshape(S // P, P).T  # [p, tt]
        mvs = np.ascontiguousarray(
            mcol[:, :, None] * vscale[n][None, None, :]
        ).astype(np.float32)
        in_maps.append({
            "xt": np.ascontiguousarray(xp),
            "wk": wkT, "wv": wvT, "wq": wqT, "wo": woT,
            "mvs": mvs, "sel": sel,
        })
    return in_maps


def kernel(x, mask, Wq, Wk, Wv, Wo, norm_const):
    in_maps = make_in_maps(x, mask, Wq, Wk, Wv, Wo, norm_const)
    nc = build()
    res = run_bass_kernel_spmd(nc, in_maps, core_ids=list(range(NCORES)))
    out = np.empty((N, S, D), dtype=np.float32)
    for c in range(NCORES):
        n, j = c // 2, c % 2
        out[n, j * SLOC : (j + 1) * SLOC, :] = res.results[c]["out"]
    return out
